# revision 31
# baseline (speedup 1.0000x reference)
"""Multihead attention (B=4, S=2048, E=1024, H=16, D=64) on 8 Trainium2 cores.

Sharding: core c = (batch b = c//2, head-half hh = c%2). Each core computes one
batch's attention for 8 heads (512 of the 1024 projection columns), producing a
partial output (row-split Wo); the host sums the two partials per batch.

Host-side prep: x is pre-transposed to xT [E, S] f16 (no on-chip transposes),
and the V-bias is folded into the output bias (bo' = bo + bv @ Wo), so the
kernel is pure matmul + softmax.

On-chip: qT/kT are [d, s]; scores [sk, sq]; softmax denominators ride along as
a ones column appended to V (M=65 matmul); exp needs no max subtraction since
scores ~ N(0,1). Normalization is deferred: fast-approx reciprocal of the
denominators, broadcast over d via K=1 PE matmuls.

Schedule: A_k, A_v upfront; then per 512-column chunk c: A_q(c) -> B(c)
(scores/exp/attnV, software-pipelined) -> A_q(c+1) -> norm(c) -> C(c) (out
projection + DMA). The interleave keeps the PE saturated so the HAM clock gate
stays at 2.4 GHz (the baseline spent 550us throttled at 1.2 GHz).

PSUM budget (8 banks): sc 2x2 + acc 2x1 + mix 2x1 = 8. The sc ring doubles as
the phase-A projection accumulator.
"""
import os
import sys

sys.path.insert(0, "/opt/trn_rl_repo")

import numpy as np

import concourse.bacc as bacc
import concourse.mybir as mybir
import concourse.tile as tile
from concourse.bass_utils import run_bass_kernel_spmd

E = 1024
H = 16
D = 64
B = 4
S = 2048
HH = E // 2          # projection cols per core
N_CORES = 8
P = 128
NCH = 4              # s-chunks of 512
CH = 512
f32 = mybir.dt.float32
f16 = mybir.dt.float16
i32 = mybir.dt.int32
AF = mybir.ActivationFunctionType

# Schraudolph fast-exp on the DVE: exp(x) ~= bitcast_f32(int32(A*x + Bc)).
# A folds the 0.125 score scale; C tuned for min RMS (~1.77%) on N(0,1) scores.
SCH_A = (1 << 23) * 1.4426950408889634 * 0.125
SCH_B = float((127 << 23) - 486500)
_DVE_EXP = os.environ.get("BASS_MHA_DVE_EXP", "0") == "1"

_cached = {}


def _build():
    mdt = f16
    nc = bacc.Bacc(None, target_bir_lowering=False)

    xqT = nc.declare_dram_parameter("xqT", [E, S], mdt, isOutput=False)
    xkT = nc.declare_dram_parameter("xkT", [E, S], mdt, isOutput=False)
    xvT = nc.declare_dram_parameter("xvT", [E, S], mdt, isOutput=False)
    wq = nc.declare_dram_parameter("wq", [P, 8, HH], mdt, isOutput=False)
    wk = nc.declare_dram_parameter("wk", [P, 8, HH], mdt, isOutput=False)
    wv = nc.declare_dram_parameter("wv", [P, 8, HH], mdt, isOutput=False)
    bq_col = nc.declare_dram_parameter("bq_col", [P, 4], f32, isOutput=False)
    bk_col = nc.declare_dram_parameter("bk_col", [P, 4], f32, isOutput=False)
    wo = nc.declare_dram_parameter("wo", [P, 4, E], mdt, isOutput=False)
    bo_col = nc.declare_dram_parameter("bo_col", [P, 8], f32, isOutput=False)
    yT = nc.declare_dram_parameter("yT", [E, S], f32, isOutput=True)

    from contextlib import ExitStack

    with tile.TileContext(nc) as tc, ExitStack() as stack:
        const = stack.enter_context(tc.tile_pool(name="const", bufs=1))
        qkv = stack.enter_context(tc.tile_pool(name="qkv", bufs=1))
        ps = stack.enter_context(tc.tile_pool(name="ps", bufs=2, space="PSUM"))
        xkvp = stack.enter_context(tc.tile_pool(name="xkv", bufs=2))
        xqp = stack.enter_context(tc.tile_pool(name="xq", bufs=2))
        exp_pool = stack.enter_context(tc.tile_pool(name="ex", bufs=5))
        oub = stack.enter_context(tc.tile_pool(name="oub", bufs=2))
        denp = stack.enter_context(tc.tile_pool(name="den", bufs=2))
        outb = stack.enter_context(tc.tile_pool(name="outb", bufs=2))

        # ---- DMA enqueue first so the queues start streaming immediately ----
        # x loads split by el-parity across the gpsimd + sync queues (~125GB/s
        # per queue); weights on the scalar queue (idle until the first exp)
        xk_sb = xkvp.tile([P, 8, S], mdt, tag="x", name="xk_sb")
        xv_sb = xkvp.tile([P, 8, S], mdt, tag="x", name="xv_sb")

        bqc = const.tile([P, 4], f32)
        bkc = const.tile([P, 4], f32)
        boc = const.tile([P, 8], f32)
        nc.sync.dma_start(out=bqc[:], in_=bq_col[:])
        nc.sync.dma_start(out=bkc[:], in_=bk_col[:])
        nc.sync.dma_start(out=boc[:], in_=bo_col[:])

        # chunk-major so A_k(c=0) can start after ~1MB instead of 4MB
        for xs, xd in ((xk_sb, xkT), (xv_sb, xvT)):
            for c in range(NCH):
                for el in range(8):
                    eng = nc.gpsimd if el % 2 == 0 else nc.sync
                    eng.dma_start(out=xs[:, el, c * CH:(c + 1) * CH],
                                  in_=xd[el * P:(el + 1) * P, c * CH:(c + 1) * CH])

        wq_t = qkv.tile([P, 8, HH], mdt)
        wk_t = qkv.tile([P, 8, HH], mdt)
        wv_t = qkv.tile([P, 8, HH], mdt)
        wo_t = qkv.tile([P, 4, E], mdt)
        nc.scalar.dma_start(out=wk_t[:], in_=wk[:])
        nc.scalar.dma_start(out=wv_t[:], in_=wv[:])
        nc.scalar.dma_start(out=wq_t[:], in_=wq[:])
        nc.scalar.dma_start(out=wo_t[:], in_=wo[:])

        # ---- constants ----
        onesf = const.tile([P, P], f32)
        nc.vector.memset(onesf[:], 1.0)
        pones_t = const.tile([P, P], mdt)      # rows 0/32/64/96: 1.0 (bcast lhsT)
        for r in (0, 32, 64, 96):
            nc.vector.tensor_copy(pones_t[r:r + 1, :], onesf[r:r + 1, :])
        vones = const.tile([P, 16, 8], mdt)    # ones column filler for vbuf
        nc.vector.memset(vones[:], 1.0)

        # persistent SBUF
        qT = qkv.tile([P, 4, S], mdt)            # [d within pair, pair, sq]
        kT = qkv.tile([P, 4, S], mdt)
        vbuf = qkv.tile([P, 16, 8, D + 1], mdt)  # [sv, s-tile, head, d|1]
        nc.vector.tensor_copy(vbuf[:, :, :, D], vones[:])

        def pe_keepalive(n, rhs_src):
            # dummy K=1 matmuls into a scratch psum bank: keeps the PE busy
            # through DMA/DVE waits so the HAM clock gate stays at 2.4 GHz
            for _ in range(n):
                pk = ps.tile([64, CH], f32, tag="mix", name="pkeep")
                nc.tensor.matmul(pk[:], lhsT=pones_t[0:1, 0:64],
                                 rhs=rhs_src, start=True, stop=True,
                                 tile_position=(0, 0))

        # warm the clock while the first x/w DMAs land (qT is junk here — the
        # results are never read)
        pe_keepalive(28, qT[0:1, 0, 0:CH])

        # ---------------- Phase A: k then v projections ----------------
        # (matmul N is capped at 512 by the 2KB fp32 PSUM bank)
        for c in range(NCH):
            cs = slice(c * CH, (c + 1) * CH)
            for u in range(4):
                pp = ps.tile([P, CH], f32, tag=("sc" if u % 2 == 0 else "mix"), name="ppk")
                for el in range(8):
                    nc.tensor.matmul(pp[:], lhsT=wk_t[:, el, u * P:(u + 1) * P],
                                     rhs=xk_sb[:, el, cs],
                                     start=(el == 0), stop=(el == 7))
                nc.vector.tensor_scalar_add(kT[:, u, cs], pp[:], bkc[:, u:u + 1])

        for st in range(16):
            pp = ps.tile([P, 8, D], f32, tag=("sc" if st % 2 == 0 else "mix"), name="ppv")
            for el in range(8):
                nc.tensor.matmul(pp[:], lhsT=xv_sb[:, el, st * P:(st + 1) * P],
                                 rhs=wv_t[:, el, :],
                                 start=(el == 0), stop=(el == 7))
            nc.vector.tensor_copy(vbuf[:, st, :, 0:D], pp[:])

        # xq chunk prefetch (sync queue; overlaps with A_k/A_v compute)
        def xq_load(c):
            cs = slice(c * CH, (c + 1) * CH)
            xq_sb = xqp.tile([P, 8, CH], mdt, tag="xq", name=f"xq{c}")
            for el in range(8):
                nc.sync.dma_start(out=xq_sb[:, el, :], in_=xqT[el * P:(el + 1) * P, cs])
            return xq_sb

        def a_q_group(c, xq_sb, u):
            cs = slice(c * CH, (c + 1) * CH)
            pp = ps.tile([P, CH], f32, tag="mix", name="ppq")
            for el in range(8):
                nc.tensor.matmul(pp[:], lhsT=wq_t[:, el, u * P:(u + 1) * P],
                                 rhs=xq_sb[:, el, :],
                                 start=(el == 0), stop=(el == 7))
            nc.vector.tensor_scalar_add(qT[:, u, cs], pp[:], bqc[:, u:u + 1])

        def norm_prs(ou_c, den_c, prs):
            # 1/den (approx; custom-DVE ops only work at partition base 0, so
            # recip the whole tile — garbage rows are unused), broadcast over d
            # via K=1 matmuls, ou *= 1/den
            den_r = denp.tile([P, 2, CH], f32, tag="denr", bufs=1, name="den_r")
            den_h = denp.tile([P, 2, CH], mdt, tag="denh", bufs=1, name="den_h")
            with nc.allow_low_precision(reason="softmax scale factors"):
                nc.vector.reciprocal_approx_fast(den_r[:], den_c[:])
                for pr in prs:
                    sl2 = pr // 2
                    for half in range(2):
                        r = 32 * ((pr % 2) * 2 + half)
                        hs = slice(64 * half, 64 * half + 64)
                        nc.vector.tensor_copy(den_h[r:r + 1, sl2, :],
                                              den_r[r:r + 1, sl2, :])
                        psb = ps.tile([64, CH], f32, tag="mix", name="psb")
                        nc.tensor.matmul(psb[:], lhsT=pones_t[r:r + 1, 0:64],
                                         rhs=den_h[r:r + 1, sl2, :],
                                         start=True, stop=True,
                                         tile_position=(r, 0))
                        nc.vector.tensor_mul(ou_c[hs, pr, :], ou_c[hs, pr, :],
                                             psb[:])

        def c_group(c, ou_c, et, tag="mix"):
            cs = slice(c * CH, (c + 1) * CH)
            po = ps.tile([P, CH], f32, tag=tag, name="po")
            for t in range(4):
                nc.tensor.matmul(po[:], lhsT=wo_t[:, t, et * P:(et + 1) * P],
                                 rhs=ou_c[:, t, :],
                                 start=(t == 0), stop=(t == 3))
            out_t = outb.tile([P, CH], f32, tag="out", name="out_t")
            nc.vector.tensor_scalar_add(out_t[:], po[:], boc[:, et:et + 1])
            eng = nc.sync if et % 2 == 0 else nc.gpsimd
            eng.dma_start(out=yT[et * P:(et + 1) * P, cs], in_=out_t[:])

        xq_tiles = {c: xq_load(c) for c in (0, 1)}
        for u in range(4):
            a_q_group(0, xq_tiles[0], u)

        # ---------------- chunk loop ----------------
        # B is Scalar(exp)-paced with ~25% PE slack; the previous chunk's
        # norm + output projection and the next chunk's q-projection are
        # emitted as filler groups inside B's matmul stream so the PE (and
        # the HAM clock) never go idle.  The scores/exp pipeline runs one
        # step ahead of attnV and crosses pr/chunk boundaries so neither
        # engine ever drains at a transition.
        ous = {}
        dens = {}
        fillers = {}
        for c in range(NCH):
            ous[c] = oub.tile([P, 4, CH], mdt, tag="ou", name=f"ou{c}")
            dens[c] = denp.tile([P, 2, CH], f32, tag="den", name=f"den{c}")
        for c in range(1, NCH):
            pou, pden = ous[c - 1], dens[c - 1]
            fillers[(c, 5)] = lambda pou=pou, pden=pden: norm_prs(pou, pden, range(4))
            for et, sl_ in enumerate((9, 13, 17, 21, 25, 27, 29, 31)):
                fillers[(c, sl_)] = (
                    lambda et=et, c=c, pou=pou: c_group(c - 1, pou, et))
        for c in range(NCH - 1):
            for u, sl_ in enumerate((1, 3, 7, 15)):
                fillers[(c, sl_)] = (
                    lambda u=u, c=c: a_q_group(
                        c + 1,
                        xq_tiles.pop(c + 1) if u == 3 else xq_tiles[c + 1], u))

        pso = {}

        def emit_scores(c, pr, i):
            cs = slice(c * CH, (c + 1) * CH)
            tiles = []
            for half in (0, 1):
                pbs = slice(64 * half, 64 * half + 64)
                psc = ps.tile([P, 2, CH], f32, tag="sc", name=f"psc{half}")
                for j in (0, 1):
                    st = 2 * i + j
                    nc.tensor.matmul(psc[:, j, :],
                                     lhsT=kT[pbs, pr, st * P:(st + 1) * P],
                                     rhs=qT[pbs, pr, cs],
                                     start=True, stop=True)
                ex = exp_pool.tile([P, 2, CH], mdt, tag="ex", name=f"ex{half}")
                if _DVE_EXP and (2 * i + half + pr) % 4 == 3:
                    # Schraudolph fast-exp on the DVE (~1.8% rms noise)
                    t32 = exp_pool.tile([P, 2, CH], i32, tag="i32", bufs=2,
                                        name="t32")
                    nc.vector.tensor_scalar(
                        out=t32[:], in0=psc[:], scalar1=SCH_A, scalar2=SCH_B,
                        op0=mybir.AluOpType.mult, op1=mybir.AluOpType.add)
                    nc.vector.tensor_copy(ex[:], t32[:].bitcast(f32))
                else:
                    nc.scalar.activation(ex[:], psc[:], AF.Exp, scale=0.125)
                tiles.append(ex)
            return tiles

        def emit_attnv(c, pr, i, tiles):
            if i == 0:
                pso[(c, pr)] = (
                    ps.tile([D + 1, CH], f32, tag="acc", name="psoA"),
                    ps.tile([D + 1, CH], f32, tag="acc", name="psoB"))
            psoA, psoB = pso[(c, pr)]
            exA, exB = tiles
            for p_, ex, hh_ in ((psoA, exA, 2 * pr), (psoB, exB, 2 * pr + 1)):
                for j in (0, 1):
                    st = 2 * i + j
                    nc.tensor.matmul(p_[:], lhsT=vbuf[:, st, hh_, :],
                                     rhs=ex[:, j, :],
                                     start=(st == 0), stop=(st == 15),
                                     skip_group_check=True)
            if i < 7:
                return
            # stash unnormalized output + denominators
            ou, den = ous[c], dens[c]
            psoA, psoB = pso.pop((c, pr))
            nc.vector.tensor_copy(ou[0:64, pr, :], psoA[0:64, :])
            nc.vector.tensor_copy(ou[64:128, pr, :], psoB[0:64, :])
            rA = 32 * ((pr % 2) * 2 + 0)
            rB = 32 * ((pr % 2) * 2 + 1)
            sl2 = pr // 2
            nc.vector.tensor_copy(den[rA:rA + 1, sl2, :], psoA[64:65, :])
            nc.vector.tensor_copy(den[rB:rB + 1, sl2, :], psoB[64:65, :])
            if c == NCH - 1:
                # last chunk: normalize per-pr so the tail C isn't serialized
                # behind a whole-chunk DVE chain
                if pr == 3:
                    # keep the clock warm through the final DVE norm chain
                    pe_keepalive(16, kT[0:1, 0, 0:CH])
                norm_prs(ou, den, [pr])

        steps = [(c, pr, i) for c in range(NCH)
                 for pr in range(4) for i in range(8)]
        pend = emit_scores(*steps[0])
        for g, (c, pr, i) in enumerate(steps):
            nxt = emit_scores(*steps[g + 1]) if g + 1 < len(steps) else None
            emit_attnv(c, pr, i, pend)
            pend = nxt
            if (c, pr * 8 + i) in fillers:
                fillers.pop((c, pr * 8 + i))()
            if i == 0 and pr == 0 and c + 2 < NCH:
                xq_tiles[c + 2] = xq_load(c + 2)

        # tail: last chunk's output projection (alternate psum rings — the
        # score ring is free by now — so the DVE bias-adds never gate the PE)
        for et in range(8):
            c_group(NCH - 1, ous[NCH - 1], et, tag=("sc" if et % 2 else "mix"))

    nc.finalize()
    return nc


def _get_nc():
    if "nc" not in _cached:
        _cached["nc"] = _build()
    return _cached["nc"]


def _in_maps(query, key, value, Wq, bq, Wk, bk, Wv, bv, Wo, bo):
    query = np.asarray(query, np.float32)
    key = np.asarray(key, np.float32)
    value = np.asarray(value, np.float32)
    Wo = np.asarray(Wo, np.float32)
    bv = np.asarray(bv, np.float32)
    bo = np.asarray(bo, np.float32)

    xT = {}
    for b in range(B):
        xT[b] = (np.ascontiguousarray(query[b].T).astype(np.float16),
                 np.ascontiguousarray(key[b].T).astype(np.float16),
                 np.ascontiguousarray(value[b].T).astype(np.float16))

    maps = []
    for c in range(N_CORES):
        b, hh = divmod(c, 2)
        sl = slice(hh * HH, (hh + 1) * HH)

        def wcols(W):
            Ws = np.asarray(W, np.float32)[:, sl]
            return np.ascontiguousarray(
                Ws.reshape(8, P, HH).transpose(1, 0, 2)).astype(np.float16)

        wo_s = Wo[sl, :]                                              # [512, E]
        wo_r = np.ascontiguousarray(
            wo_s.reshape(4, P, E).transpose(1, 0, 2)).astype(np.float16)
        # fold the V-bias through the output projection: bo' = bo + bv @ Wo
        bo_eff = bv[sl] @ wo_s + (bo if hh == 0 else 0.0)
        bo_c = np.ascontiguousarray(bo_eff.reshape(8, P).T.astype(np.float32))
        xq_b, xk_b, xv_b = xT[b]
        maps.append({
            "xqT": xq_b,
            "xkT": xk_b,
            "xvT": xv_b,
            "wq": wcols(Wq),
            "wk": wcols(Wk),
            "wv": wcols(Wv),
            "bq_col": np.ascontiguousarray(np.asarray(bq, np.float32)[sl].reshape(4, P).T),
            "bk_col": np.ascontiguousarray(np.asarray(bk, np.float32)[sl].reshape(4, P).T),
            "wo": wo_r,
            "bo_col": bo_c,
        })
    return maps


def _assemble(results):
    outs = [results[c]["yT"] for c in range(N_CORES)]
    return np.stack([(outs[2 * b] + outs[2 * b + 1]).T for b in range(B)]).astype(np.float32)


def kernel(**inputs):
    nc = _get_nc()
    maps = _in_maps(**inputs)
    r = run_bass_kernel_spmd(nc, maps, list(range(N_CORES)))
    return _assemble(r.results)


def _ensure_ntff_hook():
    """Register the axon NTFF profiling hook (missing antenv.axon_hooks shim)."""
    import contextlib
    import ctypes
    import types

    try:
        from antenv.axon_hooks import get_axon_ntff_profile_hook
        if get_axon_ntff_profile_hook() is not None:
            return
    except ImportError:
        pass

    import antenv

    holder = {}
    mod = types.ModuleType("antenv.axon_hooks")
    mod.set_axon_ntff_profile_hook = lambda h: holder.__setitem__("h", h)
    mod.get_axon_ntff_profile_hook = lambda: holder.get("h")
    sys.modules["antenv.axon_hooks"] = mod
    antenv.axon_hooks = mod

    so_path = "/opt/axon/libaxon_pjrt.so"
    lib = ctypes.CDLL(so_path)
    if not hasattr(lib, "axon_start_nrt_profile"):
        return
    lib.axon_start_nrt_profile.argtypes = [ctypes.POINTER(ctypes.c_int64), ctypes.c_size_t]
    lib.axon_start_nrt_profile.restype = ctypes.c_int64
    lib.axon_stop_nrt_profile.argtypes = [ctypes.c_char_p]
    lib.axon_stop_nrt_profile.restype = ctypes.c_int64

    @contextlib.contextmanager
    def _hook(output_dir, device_ids):
        import jax

        jax.devices()
        if device_ids:
            ids = (ctypes.c_int64 * len(device_ids))(*device_ids)
            rc = lib.axon_start_nrt_profile(ids, len(device_ids))
        else:
            rc = lib.axon_start_nrt_profile(None, 0)
        if rc != 0:
            raise RuntimeError(f"axon_start_nrt_profile rc={rc}")
        try:
            yield
        finally:
            n = lib.axon_stop_nrt_profile(str(output_dir).encode())
            if n < 0:
                raise RuntimeError(f"axon_stop_nrt_profile rc={n}")

    mod.set_axon_ntff_profile_hook(_hook)


def kernel_traced(tmpdir=None, **inputs):
    """Like kernel() but with NTFF tracing; returns (output, exec_time_ns)."""
    _ensure_ntff_hook()
    import concourse.bass_utils as bu
    bu.upload_artifacts = lambda d: d  # no artifact bucket in this container
    nc = _get_nc()
    maps = _in_maps(**inputs)
    r = run_bass_kernel_spmd(nc, maps, list(range(N_CORES)), trace=True, tmpdir=tmpdir)
    return _assemble(r.results), r.exec_time_ns


# revision 39
# speedup vs baseline: 1.0112x; 1.0112x over previous
"""Multihead attention (B=4, S=2048, E=1024, H=16, D=64) on 8 Trainium2 cores.

Sharding: core c = (batch b = c//2, head-half hh = c%2). Each core computes one
batch's attention for 8 heads (512 of the 1024 projection columns), producing a
partial output (row-split Wo); the host sums the two partials per batch.

Host-side prep: x is pre-transposed to xT [E, S] f16 (no on-chip transposes),
and the V-bias is folded into the output bias (bo' = bo + bv @ Wo), so the
kernel is pure matmul + softmax.

On-chip: qT/kT are [d, s]; scores [sk, sq]; softmax denominators ride along as
a ones column appended to V (M=65 matmul); exp needs no max subtraction since
scores ~ N(0,1). Normalization is deferred: fast-approx reciprocal of the
denominators, broadcast over d via K=1 PE matmuls.

Schedule: A_k, A_v upfront; then per 512-column chunk c: A_q(c) -> B(c)
(scores/exp/attnV, software-pipelined) -> A_q(c+1) -> norm(c) -> C(c) (out
projection + DMA). The interleave keeps the PE saturated so the HAM clock gate
stays at 2.4 GHz (the baseline spent 550us throttled at 1.2 GHz).

PSUM budget (8 banks): sc 2x2 + acc 2x1 + mix 2x1 = 8. The sc ring doubles as
the phase-A projection accumulator.
"""
import os
import sys

sys.path.insert(0, "/opt/trn_rl_repo")

import numpy as np

import concourse.bacc as bacc
import concourse.mybir as mybir
import concourse.tile as tile
from concourse.bass_utils import run_bass_kernel_spmd

E = 1024
H = 16
D = 64
B = 4
S = 2048
HH = E // 2          # projection cols per core
N_CORES = 8
P = 128
NCH = 4              # s-chunks of 512
CH = 512
f32 = mybir.dt.float32
f16 = mybir.dt.float16
i32 = mybir.dt.int32
AF = mybir.ActivationFunctionType

# Schraudolph fast-exp on the DVE, f16 flavor: exp(x) ~= bitcast_f16(int16(
# A*x + B)) in a single tensor_scalar op. A folds the 0.125 score scale; B's
# C-term tuned for min RMS (~1.77%) on N(0,1) scaled scores.
SCH_A = 1024 * 1.4426950408889634 * 0.125
SCH_B = float(15 * 1024) - 59.25
_DVE_EXP = os.environ.get("BASS_MHA_DVE_EXP", "1") == "1"

_cached = {}


def _build():
    mdt = f16
    nc = bacc.Bacc(None, target_bir_lowering=False)

    xqT = nc.declare_dram_parameter("xqT", [E, S], mdt, isOutput=False)
    xkT = nc.declare_dram_parameter("xkT", [E, S], mdt, isOutput=False)
    xvT = nc.declare_dram_parameter("xvT", [E, S], mdt, isOutput=False)
    wq = nc.declare_dram_parameter("wq", [P, 8, HH], mdt, isOutput=False)
    wk = nc.declare_dram_parameter("wk", [P, 8, HH], mdt, isOutput=False)
    wv = nc.declare_dram_parameter("wv", [P, 8, HH], mdt, isOutput=False)
    bq_col = nc.declare_dram_parameter("bq_col", [P, 4], f32, isOutput=False)
    bk_col = nc.declare_dram_parameter("bk_col", [P, 4], f32, isOutput=False)
    wo = nc.declare_dram_parameter("wo", [P, 4, E], mdt, isOutput=False)
    bo_col = nc.declare_dram_parameter("bo_col", [P, 8], f32, isOutput=False)
    yT = nc.declare_dram_parameter("yT", [E, S], f32, isOutput=True)

    from contextlib import ExitStack

    with tile.TileContext(nc) as tc, ExitStack() as stack:
        const = stack.enter_context(tc.tile_pool(name="const", bufs=1))
        qkv = stack.enter_context(tc.tile_pool(name="qkv", bufs=1))
        ps = stack.enter_context(tc.tile_pool(name="ps", bufs=2, space="PSUM"))
        xkvp = stack.enter_context(tc.tile_pool(name="xkv", bufs=2))
        xqp = stack.enter_context(tc.tile_pool(name="xq", bufs=2))
        exp_pool = stack.enter_context(tc.tile_pool(name="ex", bufs=5))
        oub = stack.enter_context(tc.tile_pool(name="oub", bufs=2))
        denp = stack.enter_context(tc.tile_pool(name="den", bufs=2))
        outb = stack.enter_context(tc.tile_pool(name="outb", bufs=2))

        # ---- DMA enqueue first so the queues start streaming immediately ----
        # x loads split by el-parity across the gpsimd + sync queues (~125GB/s
        # per queue); weights on the scalar queue (idle until the first exp)
        xk_sb = xkvp.tile([P, 8, S], mdt, tag="x", name="xk_sb")
        xv_sb = xkvp.tile([P, 8, S], mdt, tag="x", name="xv_sb")

        bqc = const.tile([P, 4], f32)
        bkc = const.tile([P, 4], f32)
        boc = const.tile([P, 8], f32)
        nc.sync.dma_start(out=bqc[:], in_=bq_col[:])
        nc.sync.dma_start(out=bkc[:], in_=bk_col[:])
        nc.sync.dma_start(out=boc[:], in_=bo_col[:])

        # chunk-major so A_k(c=0) can start after ~1MB instead of 4MB
        for xs, xd in ((xk_sb, xkT), (xv_sb, xvT)):
            for c in range(NCH):
                for el in range(8):
                    eng = nc.gpsimd if el % 2 == 0 else nc.sync
                    eng.dma_start(out=xs[:, el, c * CH:(c + 1) * CH],
                                  in_=xd[el * P:(el + 1) * P, c * CH:(c + 1) * CH])

        wq_t = qkv.tile([P, 8, HH], mdt)
        wk_t = qkv.tile([P, 8, HH], mdt)
        wv_t = qkv.tile([P, 8, HH], mdt)
        wo_t = qkv.tile([P, 4, E], mdt)
        nc.scalar.dma_start(out=wk_t[:], in_=wk[:])
        nc.scalar.dma_start(out=wv_t[:], in_=wv[:])
        nc.scalar.dma_start(out=wq_t[:], in_=wq[:])
        nc.scalar.dma_start(out=wo_t[:], in_=wo[:])

        # ---- constants ----
        onesf = const.tile([P, P], f32)
        nc.vector.memset(onesf[:], 1.0)
        pones_t = const.tile([P, P], mdt)      # rows 0/32/64/96: 1.0 (bcast lhsT)
        for r in (0, 32, 64, 96):
            nc.vector.tensor_copy(pones_t[r:r + 1, :], onesf[r:r + 1, :])
        vones = const.tile([P, 16, 8], mdt)    # ones column filler for vbuf
        nc.vector.memset(vones[:], 1.0)

        # persistent SBUF
        qT = qkv.tile([P, 4, S], mdt)            # [d within pair, pair, sq]
        kT = qkv.tile([P, 4, S], mdt)
        vbuf = qkv.tile([P, 16, 8, D + 1], mdt)  # [sv, s-tile, head, d|1]
        nc.vector.tensor_copy(vbuf[:, :, :, D], vones[:])

        def pe_keepalive(n, rhs_src):
            # dummy K=1 matmuls into a scratch psum bank: keeps the PE busy
            # through DMA/DVE waits so the HAM clock gate stays at 2.4 GHz
            for _ in range(n):
                pk = ps.tile([64, CH], f32, tag="mix", name="pkeep")
                nc.tensor.matmul(pk[:], lhsT=pones_t[0:1, 0:64],
                                 rhs=rhs_src, start=True, stop=True,
                                 tile_position=(0, 0))

        # warm the clock while the first x/w DMAs land (qT is junk here — the
        # results are never read)
        pe_keepalive(28, qT[0:1, 0, 0:CH])

        # ---------------- Phase A: k then v projections ----------------
        # (matmul N is capped at 512 by the 2KB fp32 PSUM bank)
        for c in range(NCH):
            cs = slice(c * CH, (c + 1) * CH)
            for u in range(4):
                pp = ps.tile([P, CH], f32, tag=("sc" if u % 2 == 0 else "mix"), name="ppk")
                for el in range(8):
                    nc.tensor.matmul(pp[:], lhsT=wk_t[:, el, u * P:(u + 1) * P],
                                     rhs=xk_sb[:, el, cs],
                                     start=(el == 0), stop=(el == 7))
                nc.vector.tensor_scalar_add(kT[:, u, cs], pp[:], bkc[:, u:u + 1])

        def a_v_group(st, tag=None):
            pp = ps.tile([P, 8, D], f32,
                         tag=(tag or ("sc" if st % 2 == 0 else "mix")), name="ppv")
            for el in range(8):
                nc.tensor.matmul(pp[:], lhsT=xv_sb[:, el, st * P:(st + 1) * P],
                                 rhs=wv_t[:, el, :],
                                 start=(el == 0), stop=(el == 7))
            nc.vector.tensor_copy(vbuf[:, st, :, 0:D], pp[:])

        # v s-tiles 0-9 up front; 10-15 ride inside B(0) as fillers (B(0)
        # consumes vbuf[st] at step st//2, so they stay comfortably ahead)
        for st in range(10):
            a_v_group(st)

        # xq chunk prefetch (sync queue; overlaps with A_k/A_v compute)
        def xq_load(c):
            cs = slice(c * CH, (c + 1) * CH)
            xq_sb = xqp.tile([P, 8, CH], mdt, tag="xq", name=f"xq{c}")
            for el in range(8):
                nc.sync.dma_start(out=xq_sb[:, el, :], in_=xqT[el * P:(el + 1) * P, cs])
            return xq_sb

        def a_q_group(c, xq_sb, u):
            cs = slice(c * CH, (c + 1) * CH)
            pp = ps.tile([P, CH], f32, tag="mix", name="ppq")
            for el in range(8):
                nc.tensor.matmul(pp[:], lhsT=wq_t[:, el, u * P:(u + 1) * P],
                                 rhs=xq_sb[:, el, :],
                                 start=(el == 0), stop=(el == 7))
            nc.vector.tensor_scalar_add(qT[:, u, cs], pp[:], bqc[:, u:u + 1])

        def norm_prs(ou_c, den_c, prs):
            # 1/den (approx; custom-DVE ops only work at partition base 0, so
            # recip the whole tile — garbage rows are unused), broadcast over d
            # via K=1 matmuls, ou *= 1/den
            den_r = denp.tile([P, 2, CH], f32, tag="denr", bufs=1, name="den_r")
            den_h = denp.tile([P, 2, CH], mdt, tag="denh", bufs=1, name="den_h")
            with nc.allow_low_precision(reason="softmax scale factors"):
                nc.vector.reciprocal_approx_fast(den_r[:], den_c[:])
                for pr in prs:
                    sl2 = pr // 2
                    for half in range(2):
                        r = 32 * ((pr % 2) * 2 + half)
                        hs = slice(64 * half, 64 * half + 64)
                        nc.vector.tensor_copy(den_h[r:r + 1, sl2, :],
                                              den_r[r:r + 1, sl2, :])
                        psb = ps.tile([64, CH], f32, tag="mix", name="psb")
                        nc.tensor.matmul(psb[:], lhsT=pones_t[r:r + 1, 0:64],
                                         rhs=den_h[r:r + 1, sl2, :],
                                         start=True, stop=True,
                                         tile_position=(r, 0))
                        nc.vector.tensor_mul(ou_c[hs, pr, :], ou_c[hs, pr, :],
                                             psb[:])

        def c_group(c, ou_c, et, tag="mix", last=False):
            cs = slice(c * CH, (c + 1) * CH)
            po = ps.tile([P, CH], f32, tag=tag, name="po")
            for t in range(4):
                nc.tensor.matmul(po[:], lhsT=wo_t[:, t, et * P:(et + 1) * P],
                                 rhs=ou_c[:, t, :],
                                 start=(t == 0), stop=(t == 3))
            out_t = outb.tile([P, CH], f32, tag="out", name="out_t")
            nc.vector.tensor_scalar_add(out_t[:], po[:], boc[:, et:et + 1])
            if last:  # tail: scalar queue is free after the final exp
                eng = (nc.sync, nc.gpsimd, nc.scalar)[et % 3]
            else:
                eng = nc.sync if et % 2 == 0 else nc.gpsimd
            eng.dma_start(out=yT[et * P:(et + 1) * P, cs], in_=out_t[:])

        xq_tiles = {c: xq_load(c) for c in (0, 1)}
        for u in range(4):
            a_q_group(0, xq_tiles[0], u)

        # ---------------- chunk loop ----------------
        # B is Scalar(exp)-paced with ~25% PE slack; the previous chunk's
        # norm + output projection and the next chunk's q-projection are
        # emitted as filler groups inside B's matmul stream so the PE (and
        # the HAM clock) never go idle.  The scores/exp pipeline runs one
        # step ahead of attnV and crosses pr/chunk boundaries so neither
        # engine ever drains at a transition.
        ous = {}
        dens = {}
        fillers = {}
        for c in range(NCH):
            ous[c] = oub.tile([P, 4, CH], mdt, tag="ou", name=f"ou{c}")
            dens[c] = denp.tile([P, 2, CH], f32, tag="den", name=f"den{c}")
        for c in range(1, NCH):
            pou, pden = ous[c - 1], dens[c - 1]
            fillers[(c, 5)] = lambda pou=pou, pden=pden: norm_prs(pou, pden, range(4))
            for et, sl_ in enumerate((9, 13, 17, 21, 25, 27, 29, 31)):
                fillers[(c, sl_)] = (
                    lambda et=et, c=c, pou=pou: c_group(c - 1, pou, et))
        for c in range(NCH - 1):
            aq_slots = (9, 13, 17, 21) if c == 0 else (1, 3, 7, 15)
            for u, sl_ in enumerate(aq_slots):
                fillers[(c, sl_)] = (
                    lambda u=u, c=c: a_q_group(
                        c + 1,
                        xq_tiles.pop(c + 1) if u == 3 else xq_tiles[c + 1], u))
        for k, st in enumerate(range(10, 16)):
            fillers[(0, (0, 1, 2, 3, 4, 6)[k])] = (
                lambda st=st: a_v_group(st, tag="mix"))

        pso = {}

        def emit_scores(c, pr, i):
            cs = slice(c * CH, (c + 1) * CH)
            tiles = []
            for half in (0, 1):
                pbs = slice(64 * half, 64 * half + 64)
                psc = ps.tile([P, 2, CH], f32, tag="sc", name=f"psc{half}")
                for j in (0, 1):
                    st = 2 * i + j
                    nc.tensor.matmul(psc[:, j, :],
                                     lhsT=kT[pbs, pr, st * P:(st + 1) * P],
                                     rhs=qT[pbs, pr, cs],
                                     start=True, stop=True)
                if _DVE_EXP and (2 * i + half + pr) % 8 == 3:
                    # single-op Schraudolph fast-exp on the DVE (~1.8% rms
                    # noise on 12.5% of tiles) to unload the Scalar engine:
                    # the int16 result's bit pattern IS the f16 exp
                    ex16 = exp_pool.tile([P, 2, CH], mybir.dt.int16, tag="ex",
                                         name=f"ex{half}")
                    nc.vector.tensor_scalar(
                        out=ex16[:], in0=psc[:], scalar1=SCH_A, scalar2=SCH_B,
                        op0=mybir.AluOpType.mult, op1=mybir.AluOpType.add)
                    ex = ex16.bitcast(mdt)
                else:
                    ex = exp_pool.tile([P, 2, CH], mdt, tag="ex", name=f"ex{half}")
                    nc.scalar.activation(ex[:], psc[:], AF.Exp, scale=0.125)
                tiles.append(ex)
            return tiles

        def emit_attnv(c, pr, i, tiles):
            if i == 0:
                pso[(c, pr)] = (
                    ps.tile([D + 1, CH], f32, tag="acc", name="psoA"),
                    ps.tile([D + 1, CH], f32, tag="acc", name="psoB"))
            psoA, psoB = pso[(c, pr)]
            exA, exB = tiles
            for p_, ex, hh_ in ((psoA, exA, 2 * pr), (psoB, exB, 2 * pr + 1)):
                for j in (0, 1):
                    st = 2 * i + j
                    nc.tensor.matmul(p_[:], lhsT=vbuf[:, st, hh_, :],
                                     rhs=ex[:, j, :],
                                     start=(st == 0), stop=(st == 15),
                                     skip_group_check=True)
            if i < 7:
                return
            # stash unnormalized output + denominators
            ou, den = ous[c], dens[c]
            psoA, psoB = pso.pop((c, pr))
            nc.vector.tensor_copy(ou[0:64, pr, :], psoA[0:64, :])
            nc.vector.tensor_copy(ou[64:128, pr, :], psoB[0:64, :])
            rA = 32 * ((pr % 2) * 2 + 0)
            rB = 32 * ((pr % 2) * 2 + 1)
            sl2 = pr // 2
            nc.vector.tensor_copy(den[rA:rA + 1, sl2, :], psoA[64:65, :])
            nc.vector.tensor_copy(den[rB:rB + 1, sl2, :], psoB[64:65, :])
            if c == NCH - 1:
                # last chunk: normalize per-pr so the tail C isn't serialized
                # behind a whole-chunk DVE chain
                if pr == 3:
                    # keep the clock warm through the final DVE norm chain
                    pe_keepalive(16, kT[0:1, 0, 0:CH])
                norm_prs(ou, den, [pr])

        steps = [(c, pr, i) for c in range(NCH)
                 for pr in range(4) for i in range(8)]
        pend = emit_scores(*steps[0])
        for g, (c, pr, i) in enumerate(steps):
            nxt = emit_scores(*steps[g + 1]) if g + 1 < len(steps) else None
            emit_attnv(c, pr, i, pend)
            pend = nxt
            if (c, pr * 8 + i) in fillers:
                fillers.pop((c, pr * 8 + i))()
            if i == 0 and pr == 0 and c + 2 < NCH:
                xq_tiles[c + 2] = xq_load(c + 2)

        # tail: last chunk's output projection (alternate psum rings — the
        # score ring is free by now — so the DVE bias-adds never gate the PE)
        for et in range(8):
            c_group(NCH - 1, ous[NCH - 1], et, tag=("sc" if et % 2 else "mix"),
                    last=True)

    nc.finalize()
    return nc


def _get_nc():
    if "nc" not in _cached:
        _cached["nc"] = _build()
    return _cached["nc"]


def _in_maps(query, key, value, Wq, bq, Wk, bk, Wv, bv, Wo, bo):
    query = np.asarray(query, np.float32)
    key = np.asarray(key, np.float32)
    value = np.asarray(value, np.float32)
    Wo = np.asarray(Wo, np.float32)
    bv = np.asarray(bv, np.float32)
    bo = np.asarray(bo, np.float32)

    xT = {}
    for b in range(B):
        xT[b] = (np.ascontiguousarray(query[b].T).astype(np.float16),
                 np.ascontiguousarray(key[b].T).astype(np.float16),
                 np.ascontiguousarray(value[b].T).astype(np.float16))

    maps = []
    for c in range(N_CORES):
        b, hh = divmod(c, 2)
        sl = slice(hh * HH, (hh + 1) * HH)

        def wcols(W):
            Ws = np.asarray(W, np.float32)[:, sl]
            return np.ascontiguousarray(
                Ws.reshape(8, P, HH).transpose(1, 0, 2)).astype(np.float16)

        wo_s = Wo[sl, :]                                              # [512, E]
        wo_r = np.ascontiguousarray(
            wo_s.reshape(4, P, E).transpose(1, 0, 2)).astype(np.float16)
        # fold the V-bias through the output projection: bo' = bo + bv @ Wo
        bo_eff = bv[sl] @ wo_s + (bo if hh == 0 else 0.0)
        bo_c = np.ascontiguousarray(bo_eff.reshape(8, P).T.astype(np.float32))
        xq_b, xk_b, xv_b = xT[b]
        maps.append({
            "xqT": xq_b,
            "xkT": xk_b,
            "xvT": xv_b,
            "wq": wcols(Wq),
            "wk": wcols(Wk),
            "wv": wcols(Wv),
            "bq_col": np.ascontiguousarray(np.asarray(bq, np.float32)[sl].reshape(4, P).T),
            "bk_col": np.ascontiguousarray(np.asarray(bk, np.float32)[sl].reshape(4, P).T),
            "wo": wo_r,
            "bo_col": bo_c,
        })
    return maps


def _assemble(results):
    outs = [results[c]["yT"] for c in range(N_CORES)]
    return np.stack([(outs[2 * b] + outs[2 * b + 1]).T for b in range(B)]).astype(np.float32)


def kernel(**inputs):
    nc = _get_nc()
    maps = _in_maps(**inputs)
    r = run_bass_kernel_spmd(nc, maps, list(range(N_CORES)))
    return _assemble(r.results)


def _ensure_ntff_hook():
    """Register the axon NTFF profiling hook (missing antenv.axon_hooks shim)."""
    import contextlib
    import ctypes
    import types

    try:
        from antenv.axon_hooks import get_axon_ntff_profile_hook
        if get_axon_ntff_profile_hook() is not None:
            return
    except ImportError:
        pass

    import antenv

    holder = {}
    mod = types.ModuleType("antenv.axon_hooks")
    mod.set_axon_ntff_profile_hook = lambda h: holder.__setitem__("h", h)
    mod.get_axon_ntff_profile_hook = lambda: holder.get("h")
    sys.modules["antenv.axon_hooks"] = mod
    antenv.axon_hooks = mod

    so_path = "/opt/axon/libaxon_pjrt.so"
    lib = ctypes.CDLL(so_path)
    if not hasattr(lib, "axon_start_nrt_profile"):
        return
    lib.axon_start_nrt_profile.argtypes = [ctypes.POINTER(ctypes.c_int64), ctypes.c_size_t]
    lib.axon_start_nrt_profile.restype = ctypes.c_int64
    lib.axon_stop_nrt_profile.argtypes = [ctypes.c_char_p]
    lib.axon_stop_nrt_profile.restype = ctypes.c_int64

    @contextlib.contextmanager
    def _hook(output_dir, device_ids):
        import jax

        jax.devices()
        if device_ids:
            ids = (ctypes.c_int64 * len(device_ids))(*device_ids)
            rc = lib.axon_start_nrt_profile(ids, len(device_ids))
        else:
            rc = lib.axon_start_nrt_profile(None, 0)
        if rc != 0:
            raise RuntimeError(f"axon_start_nrt_profile rc={rc}")
        try:
            yield
        finally:
            n = lib.axon_stop_nrt_profile(str(output_dir).encode())
            if n < 0:
                raise RuntimeError(f"axon_stop_nrt_profile rc={n}")

    mod.set_axon_ntff_profile_hook(_hook)


def kernel_traced(tmpdir=None, **inputs):
    """Like kernel() but with NTFF tracing; returns (output, exec_time_ns)."""
    _ensure_ntff_hook()
    import concourse.bass_utils as bu
    bu.upload_artifacts = lambda d: d  # no artifact bucket in this container
    nc = _get_nc()
    maps = _in_maps(**inputs)
    r = run_bass_kernel_spmd(nc, maps, list(range(N_CORES)), trace=True, tmpdir=tmpdir)
    return _assemble(r.results), r.exec_time_ns
